# revision 1
# baseline (speedup 1.0000x reference)
"""DILATE loss (soft-DTW shape + temporal) on 8 Trainium2 NeuronCores.

Strategy (data-parallel, per the sharding hint): the 256 independent
(batch x channel) series are sharded 32 per core; each core runs its own
128x128 DP per series with series on SBUF partitions; the scalar loss is
reduced on the host.

Per-core algorithm (gamma=0.01 makes softmin ultra-sharp, so a min-plus
DP with a pseudo-posterior gradient matches the reference closely):
  D[i,j]   = (t_i - o_j)^2
  M[i,j]   = D[i,j] + min(M[i-1,j-1], M[i-1,j], M[i,j-1])        (forward Viterbi)
  num[i,j] = D[i,j] + min(num[i,j+1], num[i+1,j], num[i+1,j+1])  (suffix Viterbi)
  E*Omega  = exp(-lam*(M - D + num - M[N,N] + womg)),  womg = -ln(Omega)/lam
  vals     = M[N,N];   tl = sum_ij (E*Omega)[i,j]
  loss     = 0.5*sum(vals)/B + 0.5*sum(tl)/(B*T*T)

Each DP row is one TT-min + one tensor_tensor_scan (min,add) on the DVE;
the D build / suffix-term / exp / reduce phases are bulk ops overlapped
across GPSIMD / ACT / DVE by the Tile scheduler.
"""
import sys
if "/opt/trn_rl_repo" not in sys.path:
    sys.path.insert(0, "/opt/trn_rl_repo")
import numpy as np
from contextlib import ExitStack

import concourse.bass as bass
import concourse.bacc as bacc
import concourse.mybir as mybir
import concourse.tile as tile
from concourse.mybir import AluOpType, ActivationFunctionType

F32 = mybir.dt.float32
S = 32          # series per core
N = 128         # DP size (= T)
LAM = 100.0     # 1/gamma
BIG = 1e30
RS = N + 1
CH = 16      # row stride in the stores (value cols + 1 guard/boundary col)
N_CORES = 8


def ap(t, off, dims):
    base = t[:]
    return bass.AP(base.tensor, base.offset + off, [base.ap[0]] + dims)


def _build_kernel():
    nc = bacc.Bacc("TRN2", target_bir_lowering=False, debug=False)
    t_d = nc.dram_tensor("t", [S, N], F32, kind="ExternalInput")
    o_d = nc.dram_tensor("o", [S, N], F32, kind="ExternalInput")
    omg_d = nc.dram_tensor("omg", [S, N * N], F32, kind="ExternalInput")
    vals_d = nc.dram_tensor("vals", [S, 1], F32, kind="ExternalOutput")
    tl_d = nc.dram_tensor("tl", [S, 1], F32, kind="ExternalOutput")

    with tile.TileContext(nc) as tc, ExitStack() as ctx:
        pool = ctx.enter_context(tc.tile_pool(name="main", bufs=1))
        t_s = pool.tile([S, N], F32, tag="t_s")
        o_s = pool.tile([S, N], F32, tag="o_s")
        D_s = pool.tile([S, RS * N], F32, tag="D_s")        # D rows; Omega DMA'd in late
        MN_s = pool.tile([S, RS * (2 * N + 1)], F32, tag="MN_s")  # M rows 0..N, then num rows
        ent_s = pool.tile([S, 2 * N], F32, tag="ent_s")     # [0:N]=fwd ent, [N:2N]=bwd entb
        stgA = pool.tile([S, CH * N], F32, tag="stgA")
        vals_s = pool.tile([S, 1], F32, tag="vals_s")
        bias_s = pool.tile([S, 1], F32, tag="bias_s")
        tl_s = pool.tile([S, 1], F32, tag="tl_s")
        tlp_s = pool.tile([S, N // CH], F32, tag="tlp_s")

        nc.sync.dma_start(t_s[:], t_d.ap())
        nc.sync.dma_start(o_s[:], o_d.ap())

        NUMOFF = RS * (N + 1)

        def m_off(r):
            return r * RS

        def n_off(r):
            return NUMOFF + (r - 1) * RS

        # guards/boundaries
        nc.gpsimd.memset(ap(D_s, N, [[RS, N], [1, 1]]), BIG)          # D col guard
        nc.gpsimd.memset(ap(MN_s, NUMOFF + N, [[RS, N], [1, 1]]), BIG)  # num col guard
        nc.vector.memset(ap(MN_s, 0, [[RS, N + 1], [1, 1]]), BIG)     # M boundary col
        nc.vector.memset(ap(MN_s, 1, [[1, N]]), BIG)                  # M row 0
        nc.vector.memset(ap(MN_s, 0, [[1, 1]]), 0.0)

        # D build on DVE, interleaved order so both DPs start early:
        # c8 (bwd needs last rows first), then c1 (fwd), then c7, c2, ...
        order = []
        lo, hi = 0, N // CH - 1
        while hi >= lo:
            order.append(lo); lo += 1
            if lo <= hi:
                order.append(hi); hi -= 1
        DCH = 4
        order = []
        lo, hi = 0, N // DCH - 1
        while hi >= lo:
            order.append(lo); lo += 1
            if lo <= hi:
                order.append(hi); hi -= 1
        for c in order:
            c0 = c * DCH
            dch = ap(D_s, c0 * RS, [[RS, DCH], [1, N]])
            t_ch = ap(t_s, c0, [[1, DCH], [0, N]])
            o_ch = ap(o_s, 0, [[0, DCH], [1, N]])
            nc.gpsimd.tensor_tensor(dch, t_ch, o_ch, AluOpType.subtract)
            nc.scalar.activation(dch, dch, ActivationFunctionType.Square)

        # two independent DP chains (fwd ascending, suffix descending) —
        # kept separate so their instruction streams fill each other's
        # pipeline bubbles on the DVE.
        nc.gpsimd.memset(ap(ent_s, N, [[1, N]]), BIG)
        nc.gpsimd.memset(ap(ent_s, 2 * N - 1, [[1, 1]]), 0.0)
        for r in range(N, 0, -1):
            if r < N:
                nc.vector.tensor_tensor(
                    ap(ent_s, N, [[1, N]]),
                    ap(MN_s, n_off(r + 1), [[1, N]]),
                    ap(MN_s, n_off(r + 1) + 1, [[1, N]]),
                    AluOpType.min)
            nc.vector.tensor_tensor_scan(
                ap(MN_s, n_off(r) + N - 1, [[-1, N]]),
                ap(ent_s, 2 * N - 1, [[-1, N]]),
                ap(D_s, (r - 1) * RS + N - 1, [[-1, N]]),
                BIG, AluOpType.min, AluOpType.add)
        for r in range(1, N + 1):
            nc.vector.tensor_tensor(
                ap(ent_s, 0, [[1, N]]),
                ap(MN_s, m_off(r - 1), [[1, N]]), ap(MN_s, m_off(r - 1) + 1, [[1, N]]),
                AluOpType.min)
            nc.vector.tensor_tensor_scan(
                ap(MN_s, m_off(r) + 1, [[1, N]]), ap(ent_s, 0, [[1, N]]),
                ap(D_s, (r - 1) * RS, [[1, N]]),
                BIG, AluOpType.min, AluOpType.add)

        # vals = M[N,N]; bias = +lam*M[N,N]  (before womg is folded into M!)
        nc.vector.tensor_copy(vals_s[:], ap(MN_s, m_off(N) + N, [[1, 1]]))
        nc.vector.tensor_scalar(bias_s[:], vals_s[:], LAM, None, AluOpType.mult)

        # fold womg (= -ln(Omega)/lam) into M, chunk-staged through SBUF on
        # gpsimd -- runs during the row phase as fwd rows complete
        for ci in range(N // CH):
            c0 = ci * CH
            stg = stgA
            nc.sync.dma_start(stg[:], bass.AP(omg_d, c0 * N, [[N * N, S], [1, CH * N]]))
            mch = ap(MN_s, m_off(c0 + 1) + 1, [[RS, CH], [1, N]])
            nc.gpsimd.tensor_tensor(mch, mch, ap(stg, 0, [[N, CH], [1, N]]),
                                    AluOpType.add)

        # epilogue, chunked: arg = (M + womg) + num - D;
        # E*Omega = exp(-lam*arg + lam*MNN) with the chunk partial sum taken
        # directly from the ACT Exp's accum_out (no multiply, no DVE reduce)
        for ci in range(N // CH - 1, -1, -1):
            c0 = ci * CH
            eng = nc.gpsimd if ci <= 3 else nc.vector  # gps is free after its DP
            mch = ap(MN_s, m_off(c0 + 1) + 1, [[RS, CH], [1, N]])
            dch = ap(D_s, c0 * RS, [[RS, CH], [1, N]])
            nch = ap(MN_s, n_off(c0 + 1), [[RS, CH], [1, N]])
            eng.tensor_tensor(mch, mch, nch, AluOpType.add)        # + num
            eng.tensor_tensor(mch, mch, dch, AluOpType.subtract)   # - D
            nc.scalar.activation(nch, mch, ActivationFunctionType.Exp,
                                 bias=bias_s[:], scale=-LAM,
                                 accum_out=tlp_s[:, ci:ci + 1])
        nc.vector.tensor_reduce(tl_s[:], tlp_s[:], mybir.AxisListType.X, AluOpType.add)

        nc.sync.dma_start(vals_d.ap(), vals_s[:])
        nc.sync.dma_start(tl_d.ap(), tl_s[:])

    nc.compile()
    return nc



_NC_CACHE = None
_OMG_CACHE = None


def _get_nc():
    global _NC_CACHE
    if _NC_CACHE is None:
        _NC_CACHE = _build_kernel()
    return _NC_CACHE


def _womg():
    global _OMG_CACHE
    if _OMG_CACHE is None:
        idx = np.arange(1, N + 1, dtype=np.float64)
        om2d = ((idx[:, None] - idx[None, :]) ** 2).reshape(N * N)
        w = np.where(om2d == 0.0, BIG, -np.log(np.maximum(om2d, 1e-30)) / LAM)
        _OMG_CACHE = np.ascontiguousarray(
            np.broadcast_to(w.astype(np.float32), (S, N * N)))
    return _OMG_CACHE


_EXEC_CACHE = None


def _get_exec():
    """Build the sharded jitted executable once (mirrors bass2jax's
    run_bass_via_pjrt multi-core path) and keep the big constant omg input
    resident on the devices."""
    global _EXEC_CACHE
    if _EXEC_CACHE is not None:
        return _EXEC_CACHE
    import jax
    import concourse.mybir as _mybir
    from jax.sharding import Mesh, PartitionSpec, NamedSharding
    from jax.experimental.shard_map import shard_map
    from concourse.bass2jax import (
        _bass_exec_p, install_neuronx_cc_hook, partition_id_tensor)

    nc = _get_nc()
    install_neuronx_cc_hook()
    partition_name = nc.partition_id_tensor.name if nc.partition_id_tensor else None
    in_names, out_names, out_avals, zero_outs = [], [], [], []
    for alloc in nc.m.functions[0].allocations:
        if not isinstance(alloc, _mybir.MemoryLocationSet):
            continue
        name = alloc.memorylocations[0].name
        if alloc.kind == "ExternalInput":
            if name != partition_name:
                in_names.append(name)
        elif alloc.kind == "ExternalOutput":
            shape = tuple(alloc.tensor_shape)
            dtype = _mybir.dt.np(alloc.dtype)
            out_names.append(name)
            out_avals.append(jax.core.ShapedArray(shape, dtype))
            zero_outs.append(np.zeros(shape, dtype))
    n_params = len(in_names)
    all_in_names = list(in_names) + list(out_names)
    if partition_name is not None:
        all_in_names.append(partition_name)
    donate = tuple(range(n_params, n_params + len(out_names)))

    def _body(*args):
        operands = list(args)
        if partition_name is not None:
            operands.append(partition_id_tensor())
        return tuple(_bass_exec_p.bind(
            *operands,
            out_avals=tuple(out_avals),
            in_names=tuple(all_in_names),
            out_names=tuple(out_names),
            lowering_input_output_aliases=(),
            sim_require_finite=True,
            sim_require_nnan=True,
            nc=nc,
        ))

    devices = jax.devices()[:N_CORES]
    mesh = Mesh(np.asarray(devices), ("core",))
    in_specs = (PartitionSpec("core"),) * (n_params + len(out_names))
    out_specs = (PartitionSpec("core"),) * len(out_names)
    sharded = jax.jit(
        shard_map(_body, mesh=mesh, in_specs=in_specs, out_specs=out_specs,
                  check_rep=False),
        donate_argnums=donate, keep_unused=True)
    shard = NamedSharding(mesh, PartitionSpec("core"))
    omg_dev = jax.device_put(
        np.concatenate([_womg()] * N_CORES, axis=0), shard)
    _EXEC_CACHE = (sharded, in_names, out_names, zero_outs, shard, omg_dev)
    return _EXEC_CACHE


def kernel(outputs, targets):
    """outputs, targets: [64, 128, 4] float32 -> scalar float32 loss."""
    sharded, in_names, out_names, zero_outs, shard, omg_dev = _get_exec()
    outputs = np.asarray(outputs, np.float32)
    targets = np.asarray(targets, np.float32)
    B, T, C = outputs.shape
    t = np.ascontiguousarray(np.transpose(targets, (0, 2, 1)).reshape(B * C, T))
    o = np.ascontiguousarray(np.transpose(outputs, (0, 2, 1)).reshape(B * C, T))
    by_name = {"t": t, "o": o, "omg": omg_dev}
    concat_in = [by_name[name] for name in in_names]
    concat_zeros = [
        np.zeros((N_CORES * z.shape[0], *z.shape[1:]), z.dtype) for z in zero_outs
    ]
    out_arrs = sharded(*concat_in, *concat_zeros)
    outs = {name: np.asarray(out_arrs[i]) for i, name in enumerate(out_names)}
    vals = outs["vals"][:, 0]
    tl = outs["tl"][:, 0]
    loss = 0.5 * (vals.sum(dtype=np.float64) / B) + \
           0.5 * (tl.sum(dtype=np.float64) / (B * T * T))
    return np.float32(loss)



# revision 2
# speedup vs baseline: 1.4895x; 1.4895x over previous
"""DILATE loss (soft-DTW shape + temporal) on 8 Trainium2 NeuronCores.

Strategy: the 256 (batch x channel) series are sharded 32 per core. Each
core runs a BANDED min-plus DP (band half-width W around the diagonal;
gamma=0.01 makes softmin ~min and the soft alignment posterior razor
sharp, so the band is lossless to well under the tolerance):

  D[i,j]   = (t_i - o_j)^2               (banded, j in [i-W, i+W])
  M[i,j]   = D + min(M[i-1,j-1], M[i-1,j], M[i,j-1])       (forward)
  num[i,j] = D + min(num[i+1,j+1], num[i+1,j], num[i,j+1]) (suffix)
  E*Omega  = exp(-lam*(M + num - D + womg - M[N,N])), womg = -ln(Om)/lam
  loss     = 0.5*sum(M[N,N])/B + 0.5*sum(E*Omega)/(B*T*T)

Layout: banded rows are spread over all 128 SBUF partitions as
(series s, row-group g): partition s+32g holds rows 32g..32g+31. Bulk
passes (D build, epilogue) then cost 1/4 the free-size. The DP rows run
on the 32 partitions of their group; group boundaries are crossed with
tiny SBUF->SBUF DMA hops.

Engine schedule: the fwd chain starts on DVE while the suffix chain
starts on GPSIMD; the chains swap engines mid-flight (FA/BA splits) so
both engines stay saturated. The epilogue is chunked so it pipelines
into the DP tail; Exp with accumulate runs on ACT.
"""
import sys
if "/opt/trn_rl_repo" not in sys.path:
    sys.path.insert(0, "/opt/trn_rl_repo")
import numpy as np
from contextlib import ExitStack

import concourse.bass as bass
import concourse.bacc as bacc
import concourse.mybir as mybir
import concourse.tile as tile
from concourse.mybir import AluOpType, ActivationFunctionType

F32 = mybir.dt.float32
S = 32            # series per core
N = 128           # T
G = 4             # partition row-groups
LR = N // G       # rows per group (32)
W = 24            # band half-width
Wb = 2 * W + 1    # banded row width (49)
RS = Wb + 1       # row stride in M/num tiles (one guard col)
OW = LR + Wb - 1  # o_grouped width (80)
LAM = 100.0
BIG = 1e30
SENT = 1e15       # o padding sentinel -> D ~ 1e30 outside the valid square
N_CORES = 8

FA = 50           # fwd rows 0..FA-1 on DVE, rest on Pool
BA = 34           # bwd rows 127..128-BA on Pool, rest on DVE
DCH = 8           # D-build chunk rows
ECH = 8           # epilogue chunk rows


def gap(t, p0, pn, off, dims):
    """AP on partitions [p0, p0+pn) of tile t, free offset off, free dims."""
    base = t[p0:p0 + pn, 0:1]
    return bass.AP(base.tensor, base.offset + off, [base.ap[0]] + dims)


def _build_kernel():
    nc = bacc.Bacc("TRN2", target_bir_lowering=False, debug=False)
    tg_d = nc.dram_tensor("tg", [G * S, LR], F32, kind="ExternalInput")
    og_d = nc.dram_tensor("og", [G * S, OW], F32, kind="ExternalInput")
    wom_d = nc.dram_tensor("wom", [G * S, Wb], F32, kind="ExternalInput")
    vals_d = nc.dram_tensor("vals", [S, 1], F32, kind="ExternalOutput")
    tl_d = nc.dram_tensor("tl", [G * S, 1], F32, kind="ExternalOutput")

    NP = G * S  # 128 partitions

    with tile.TileContext(nc) as tc, ExitStack() as ctx:
        pool = ctx.enter_context(tc.tile_pool(name="main", bufs=1))
        tg = pool.tile([NP, LR], F32, tag="tg")
        og = pool.tile([NP, OW], F32, tag="og")
        wom = pool.tile([NP, Wb], F32, tag="wom")
        Dg = pool.tile([NP, LR * Wb], F32, tag="Dg")
        Mt = pool.tile([NP, (LR + 1) * RS], F32, tag="Mt")
        Nt = pool.tile([NP, (LR + 1) * RS], F32, tag="Nt")
        entF = pool.tile([NP, Wb], F32, tag="entF")
        entB = pool.tile([NP, Wb], F32, tag="entB")
        Xg = pool.tile([NP, LR * Wb], F32, tag="Xg")
        Yg = pool.tile([NP, LR * Wb], F32, tag="Yg")
        bias = pool.tile([NP, 1], F32, tag="bias")
        tlp = pool.tile([NP, LR // ECH], F32, tag="tlp")
        tls = pool.tile([NP, 1], F32, tag="tls")

        # ---- init: guards and virtual boundary rows -------------------
        # M right-guard col + num left-guard col (all slots, all parts)
        nc.vector.memset(gap(Mt, 0, NP, Wb, [[RS, LR + 1], [1, 1]]), BIG)
        nc.vector.memset(gap(Nt, 0, NP, 0, [[RS, LR + 1], [1, 1]]), BIG)
        # fwd virtual row -1 on group 0: BIG except k=W (the DP origin)
        nc.vector.memset(Mt[0:S, 0:Wb], BIG)
        nc.vector.memset(Mt[0:S, W:W + 1], 0.0)
        # bwd virtual row 128 on group 3 (slot LR): BIG except k=W
        nc.gpsimd.memset(Nt[(G - 1) * S:NP, LR * RS + 1:LR * RS + 1 + Wb], BIG)
        nc.gpsimd.memset(Nt[(G - 1) * S:NP, LR * RS + 1 + W:LR * RS + 2 + W], 0.0)

        # ---- input DMAs ----------------------------------------------
        nc.sync.dma_start(tg[:], tg_d.ap())
        nc.sync.dma_start(og[:], og_d.ap())
        nc.sync.dma_start(wom[:], wom_d.ap())

        # ---- D build: D = (t_bcast - o_sliding)^2, chunked ------------
        # chunk order A(0:8) C(8:16) on DVE, B(24:32) D(16:24) on Pool so
        # both chain heads unblock early; Square on ACT.
        def d_chunk(eng, c0):
            dch = gap(Dg, 0, NP, c0 * Wb, [[Wb, DCH], [1, Wb]])
            t_ch = gap(tg, 0, NP, c0, [[1, DCH], [0, Wb]])
            o_ch = gap(og, 0, NP, c0, [[1, DCH], [1, Wb]])
            eng.tensor_tensor(dch, t_ch, o_ch, AluOpType.subtract)
            return dch

        sq = []
        sq.append(d_chunk(nc.vector, 0))
        sq.append(d_chunk(nc.vector, DCH))
        sqp = []
        sqp.append(d_chunk(nc.gpsimd, LR - DCH))
        sqp.append(d_chunk(nc.gpsimd, LR - 2 * DCH))
        for dch in (sq[0], sq[1], sqp[0], sqp[1]):
            nc.scalar.activation(dch, dch, ActivationFunctionType.Square)

        # ---- DP rows --------------------------------------------------
        def fwd_row(r):
            eng = nc.vector if r < FA else nc.gpsimd
            g, rho = r // LR, r % LR
            p0 = g * S
            eng.tensor_tensor(
                gap(entF, p0, S, 0, [[1, Wb]]),
                gap(Mt, p0, S, rho * RS, [[1, Wb]]),
                gap(Mt, p0, S, rho * RS + 1, [[1, Wb]]),
                AluOpType.min)
            eng.tensor_tensor_scan(
                gap(Mt, p0, S, (rho + 1) * RS, [[1, Wb]]),
                gap(entF, p0, S, 0, [[1, Wb]]),
                gap(Dg, p0, S, rho * Wb, [[1, Wb]]),
                BIG, AluOpType.min, AluOpType.add)
            if rho == LR - 1 and g < G - 1:
                # hop: row r becomes slot 0 of group g+1
                nc.sync.dma_start(
                    gap(Mt, p0 + S, S, 0, [[1, RS]]),
                    gap(Mt, p0, S, LR * RS, [[1, RS]]))

        def bwd_row(i):
            eng = nc.gpsimd if i >= N - BA else nc.vector
            g, rho = i // LR, i % LR
            p0 = g * S
            eng.tensor_tensor(
                gap(entB, p0, S, 0, [[1, Wb]]),
                gap(Nt, p0, S, (rho + 1) * RS, [[1, Wb]]),
                gap(Nt, p0, S, (rho + 1) * RS + 1, [[1, Wb]]),
                AluOpType.min)
            eng.tensor_tensor_scan(
                gap(Nt, p0, S, rho * RS + Wb, [[-1, Wb]]),
                gap(entB, p0, S, Wb - 1, [[-1, Wb]]),
                gap(Dg, p0, S, rho * Wb + Wb - 1, [[-1, Wb]]),
                BIG, AluOpType.min, AluOpType.add)
            if rho == 0 and g > 0:
                # hop: row i becomes slot LR of group g-1 (issued on ACT
                # queue to keep SP's queue in temporal order)
                nc.scalar.dma_start(
                    gap(Nt, p0 - S, S, LR * RS, [[1, RS]]),
                    gap(Nt, p0, S, 0, [[1, RS]]))

        # fwd head on DVE / bwd head on Pool run concurrently; then swap.
        for r in range(0, FA):
            fwd_row(r)
        # Y = womg - D (DVE, fills the gap while Pool works the fwd tail)
        nc.vector.tensor_tensor(
            gap(Yg, 0, NP, 0, [[Wb, LR], [1, Wb]]),
            gap(wom, 0, NP, 0, [[0, LR], [1, Wb]]),
            gap(Dg, 0, NP, 0, [[Wb, LR], [1, Wb]]),
            AluOpType.subtract)
        for i in range(N - 1, N - 1 - BA, -1):
            bwd_row(i)
        for r in range(FA, N):
            fwd_row(r)
        for i in range(N - 1 - BA, -1, -1):
            bwd_row(i)

        # ---- bias = +lam * M[N,N], replicated to all groups -----------
        p3 = (G - 1) * S
        nc.vector.tensor_scalar(
            bias[p3:NP, 0:1], gap(Mt, p3, S, LR * RS + W, [[1, 1]]),
            LAM, None, AluOpType.mult)
        for g in range(G - 1):
            nc.sync.dma_start(bias[g * S:(g + 1) * S, 0:1], bias[p3:NP, 0:1])
        nc.sync.dma_start(vals_d.ap(), gap(Mt, p3, S, LR * RS + W, [[1, 1]]))

        # ---- epilogue: X = M + num; X += Y; E*Om = Exp(-lam X + bias) -
        for ci in range(LR // ECH):
            c0 = ci * ECH
            xch = gap(Xg, 0, NP, c0 * Wb, [[Wb, ECH], [1, Wb]])
            nc.vector.tensor_tensor(
                xch,
                gap(Mt, 0, NP, (c0 + 1) * RS, [[RS, ECH], [1, Wb]]),
                gap(Nt, 0, NP, c0 * RS + 1, [[RS, ECH], [1, Wb]]),
                AluOpType.add)
            nc.vector.tensor_tensor(
                xch, xch, gap(Yg, 0, NP, c0 * Wb, [[Wb, ECH], [1, Wb]]),
                AluOpType.add)
            nc.scalar.activation(
                gap(Yg, 0, NP, c0 * Wb, [[Wb, ECH], [1, Wb]]), xch,
                ActivationFunctionType.Exp,
                bias=bias[:, 0:1], scale=-LAM,
                accum_out=tlp[:, ci:ci + 1])
        nc.vector.tensor_reduce(tls[:], tlp[:], mybir.AxisListType.X,
                                AluOpType.add)
        nc.sync.dma_start(tl_d.ap(), tls[:])

    nc.compile()
    return nc


_NC_CACHE = None


def _get_nc():
    global _NC_CACHE
    if _NC_CACHE is None:
        _NC_CACHE = _build_kernel()
    return _NC_CACHE


def _host_inputs(outputs, targets):
    """Full inputs -> per-core grouped/banded host arrays, concatenated."""
    outputs = np.asarray(outputs, np.float32)
    targets = np.asarray(targets, np.float32)
    B, T, C = outputs.shape
    t = np.ascontiguousarray(
        np.transpose(targets, (0, 2, 1)).reshape(B * C, T))
    o = np.ascontiguousarray(
        np.transpose(outputs, (0, 2, 1)).reshape(B * C, T))
    # grouped t: partition s+32g holds t[s, 32g:32g+32]
    tg = (t.reshape(N_CORES, S, G, LR).transpose(0, 2, 1, 3)
          .reshape(N_CORES * G * S, LR))
    # padded, grouped o: partition s+32g holds opad[s, 32g : 32g+OW]
    opad = np.full((B * C, T + 2 * W), SENT, np.float32)
    opad[:, W:W + T] = o
    og = np.empty((N_CORES, G, S, OW), np.float32)
    opad_c = opad.reshape(N_CORES, S, T + 2 * W)
    for g in range(G):
        og[:, g, :, :] = opad_c[:, :, g * LR:g * LR + OW]
    og = og.reshape(N_CORES * G * S, OW)
    return np.ascontiguousarray(tg), np.ascontiguousarray(og)


def _womg():
    k = np.arange(Wb, dtype=np.float64)
    om = (W - k) ** 2
    w = np.where(om == 0.0, BIG, -np.log(np.maximum(om, 1e-30)) / LAM)
    w = np.broadcast_to(w.astype(np.float32), (N_CORES * G * S, Wb))
    return np.ascontiguousarray(w)


_EXEC_CACHE = None


def _get_exec():
    """Build the sharded jitted executable once (mirrors bass2jax's
    run_bass_via_pjrt multi-core path)."""
    global _EXEC_CACHE
    if _EXEC_CACHE is not None:
        return _EXEC_CACHE
    import jax
    import concourse.mybir as _mybir
    from jax.sharding import Mesh, PartitionSpec, NamedSharding
    from jax.experimental.shard_map import shard_map
    from concourse.bass2jax import (
        _bass_exec_p, install_neuronx_cc_hook, partition_id_tensor)

    nc = _get_nc()
    install_neuronx_cc_hook()
    partition_name = nc.partition_id_tensor.name if nc.partition_id_tensor else None
    in_names, out_names, out_avals, zero_outs = [], [], [], []
    for alloc in nc.m.functions[0].allocations:
        if not isinstance(alloc, _mybir.MemoryLocationSet):
            continue
        name = alloc.memorylocations[0].name
        if alloc.kind == "ExternalInput":
            if name != partition_name:
                in_names.append(name)
        elif alloc.kind == "ExternalOutput":
            shape = tuple(alloc.tensor_shape)
            dtype = _mybir.dt.np(alloc.dtype)
            out_names.append(name)
            out_avals.append(jax.core.ShapedArray(shape, dtype))
            zero_outs.append(np.zeros(shape, dtype))
    n_params = len(in_names)
    all_in_names = list(in_names) + list(out_names)
    if partition_name is not None:
        all_in_names.append(partition_name)
    donate = tuple(range(n_params, n_params + len(out_names)))

    def _body(*args):
        operands = list(args)
        if partition_name is not None:
            operands.append(partition_id_tensor())
        return tuple(_bass_exec_p.bind(
            *operands,
            out_avals=tuple(out_avals),
            in_names=tuple(all_in_names),
            out_names=tuple(out_names),
            lowering_input_output_aliases=(),
            sim_require_finite=True,
            sim_require_nnan=True,
            nc=nc,
        ))

    devices = jax.devices()[:N_CORES]
    mesh = Mesh(np.asarray(devices), ("core",))
    in_specs = (PartitionSpec("core"),) * (n_params + len(out_names))
    out_specs = (PartitionSpec("core"),) * len(out_names)
    sharded = jax.jit(
        shard_map(_body, mesh=mesh, in_specs=in_specs, out_specs=out_specs,
                  check_rep=False),
        donate_argnums=donate, keep_unused=True)
    shard = NamedSharding(mesh, PartitionSpec("core"))
    wom_dev = jax.device_put(_womg(), shard)
    _EXEC_CACHE = (sharded, in_names, out_names, zero_outs, wom_dev)
    return _EXEC_CACHE


def kernel(outputs, targets):
    """outputs, targets: [64, 128, 4] float32 -> scalar float32 loss."""
    sharded, in_names, out_names, zero_outs, wom_dev = _get_exec()
    B, T, C = np.asarray(outputs).shape
    tg, og = _host_inputs(outputs, targets)
    by_name = {"tg": tg, "og": og, "wom": wom_dev}
    concat_in = [by_name[name] for name in in_names]
    concat_zeros = [
        np.zeros((N_CORES * z.shape[0], *z.shape[1:]), z.dtype)
        for z in zero_outs
    ]
    out_arrs = sharded(*concat_in, *concat_zeros)
    outs = {name: np.asarray(out_arrs[i]) for i, name in enumerate(out_names)}
    vals = outs["vals"][:, 0]
    tl = outs["tl"][:, 0]
    loss = 0.5 * (vals.sum(dtype=np.float64) / B) + \
           0.5 * (tl.sum(dtype=np.float64) / (B * T * T))
    return np.float32(loss)


# revision 4
# speedup vs baseline: 1.6398x; 1.1009x over previous
"""DILATE loss (soft-DTW shape + temporal) on 8 Trainium2 NeuronCores.

Strategy: the 256 (batch x channel) series are sharded 32 per core. Each
core runs a BANDED min-plus DP (band half-width W around the diagonal;
gamma=0.01 makes softmin ~min and the soft alignment posterior razor
sharp, so the band is lossless to well under the tolerance):

  D[i,j]   = (t_i - o_j)^2               (banded, j in [i-W, i+W])
  M[i,j]   = D + min(M[i-1,j-1], M[i-1,j], M[i,j-1])       (forward)
  num[i,j] = D + min(num[i+1,j+1], num[i+1,j], num[i,j+1]) (suffix)
  E*Omega  = exp(-lam*(M + num - D + womg - M[N,N])), womg = -ln(Om)/lam
  loss     = 0.5*sum(M[N,N])/B + 0.5*sum(E*Omega)/(B*T*T)

Layout: banded rows are spread over all 128 SBUF partitions as
(series s, row-group g): partition s+32g holds rows 32g..32g+31. Bulk
passes (D build, epilogue) then cost 1/4 the free-size. The DP rows run
on the 32 partitions of their group; group boundaries are crossed with
tiny SBUF->SBUF DMA hops.

Engine schedule: the fwd chain starts on DVE while the suffix chain
starts on GPSIMD; the chains swap engines mid-flight (FA/BA splits) so
both engines stay saturated. The epilogue is chunked so it pipelines
into the DP tail; Exp with accumulate runs on ACT.
"""
import sys
if "/opt/trn_rl_repo" not in sys.path:
    sys.path.insert(0, "/opt/trn_rl_repo")
import numpy as np
from contextlib import ExitStack

import concourse.bass as bass
import concourse.bacc as bacc
import concourse.mybir as mybir
import concourse.tile as tile
from concourse.mybir import AluOpType, ActivationFunctionType

F32 = mybir.dt.float32
S = 32            # series per core
N = 128           # T
G = 4             # partition row-groups
LR = N // G       # rows per group (32)
W = 24            # band half-width
Wb = 2 * W + 1    # banded row width (49)
RS = Wb + 1       # row stride in M/num tiles (one guard col)
OW = LR + Wb - 1  # o_grouped width (80)
LAM = 100.0
BIG = 1e30
SENT = 1e15       # o padding sentinel -> D ~ 1e30 outside the valid square
N_CORES = 8

DCH = 8           # D-build chunk rows
ECH = 8           # epilogue chunk rows


def gap(t, p0, pn, off, dims):
    """AP on partitions [p0, p0+pn) of tile t, free offset off, free dims."""
    base = t[p0:p0 + pn, 0:1]
    return bass.AP(base.tensor, base.offset + off, [base.ap[0]] + dims)


def _build_kernel():
    nc = bacc.Bacc("TRN2", target_bir_lowering=False, debug=False)
    tg_d = nc.dram_tensor("tg", [G * S, LR], F32, kind="ExternalInput")
    og_d = nc.dram_tensor("og", [G * S, OW], F32, kind="ExternalInput")
    wom_d = nc.dram_tensor("wom", [G * S, Wb], F32, kind="ExternalInput")
    vals_d = nc.dram_tensor("vals", [S, 1], F32, kind="ExternalOutput")
    tl_d = nc.dram_tensor("tl", [G * S, 1], F32, kind="ExternalOutput")

    NP = G * S  # 128 partitions

    with tile.TileContext(nc) as tc, ExitStack() as ctx:
        pool = ctx.enter_context(tc.tile_pool(name="main", bufs=1))
        tg = pool.tile([NP, LR], F32, tag="tg")
        og = pool.tile([NP, OW], F32, tag="og")
        wom = pool.tile([NP, Wb], F32, tag="wom")
        Dg = pool.tile([NP, LR * Wb], F32, tag="Dg")
        Mt = pool.tile([NP, (LR + 1) * RS], F32, tag="Mt")
        Nt = pool.tile([NP, (LR + 1) * RS], F32, tag="Nt")
        entF = pool.tile([NP, Wb], F32, tag="entF")
        entB = pool.tile([NP, Wb], F32, tag="entB")
        Xg = pool.tile([NP, LR * Wb], F32, tag="Xg")
        Yg = pool.tile([NP, LR * Wb], F32, tag="Yg")
        bias = pool.tile([NP, 1], F32, tag="bias")
        tlp = pool.tile([NP, LR // ECH], F32, tag="tlp")
        tls = pool.tile([NP, 1], F32, tag="tls")

        # ---- init: guards and virtual boundary rows -------------------
        # M right-guard col + num left-guard col (all slots, all parts)
        nc.vector.memset(gap(Mt, 0, NP, Wb, [[RS, LR + 1], [1, 1]]), BIG)
        nc.vector.memset(gap(Nt, 0, NP, 0, [[RS, LR + 1], [1, 1]]), BIG)
        # fwd virtual row -1 on group 0: BIG except k=W (the DP origin)
        nc.vector.memset(Mt[0:S, 0:Wb], BIG)
        nc.vector.memset(Mt[0:S, W:W + 1], 0.0)
        # bwd virtual row 128 on group 3 (slot LR): BIG except k=W
        nc.gpsimd.memset(Nt[(G - 1) * S:NP, LR * RS + 1:LR * RS + 1 + Wb], BIG)
        nc.gpsimd.memset(Nt[(G - 1) * S:NP, LR * RS + 1 + W:LR * RS + 2 + W], 0.0)

        # ---- input DMAs ----------------------------------------------
        nc.sync.dma_start(tg[:], tg_d.ap())
        nc.sync.dma_start(og[:], og_d.ap())
        nc.sync.dma_start(wom[:], wom_d.ap())

        # warm the ACT function table during otherwise-idle time
        nc.scalar.activation(bias[:, 0:1], bias[:, 0:1],
                             ActivationFunctionType.Exp, scale=0.0)

        # ---- D build: D = (t_bcast - o_sliding)^2 on Pool, Sq on ACT --
        # chunk order serves both chain heads (fwd wants low rho, bwd
        # wants high rho first).
        def d_chunk(c0):
            dch = gap(Dg, 0, NP, c0 * Wb, [[Wb, DCH], [1, Wb]])
            t_ch = gap(tg, 0, NP, c0, [[1, DCH], [0, Wb]])
            o_ch = gap(og, 0, NP, c0, [[1, DCH], [1, Wb]])
            nc.gpsimd.tensor_tensor(dch, t_ch, o_ch, AluOpType.subtract)
            nc.scalar.activation(dch, dch, ActivationFunctionType.Square)

        for c0 in (0, LR - DCH, DCH, LR - 2 * DCH):
            d_chunk(c0)

        # ---- DP rows: both chains interleaved 1:1 on DVE --------------
        def fwd_row(r):
            g, rho = r // LR, r % LR
            p0 = g * S
            nc.vector.tensor_tensor(
                gap(entF, p0, S, 0, [[1, Wb]]),
                gap(Mt, p0, S, rho * RS, [[1, Wb]]),
                gap(Mt, p0, S, rho * RS + 1, [[1, Wb]]),
                AluOpType.min)
            nc.vector.tensor_tensor_scan(
                gap(Mt, p0, S, (rho + 1) * RS, [[1, Wb]]),
                gap(entF, p0, S, 0, [[1, Wb]]),
                gap(Dg, p0, S, rho * Wb, [[1, Wb]]),
                BIG, AluOpType.min, AluOpType.add)
            if rho == LR - 1 and g < G - 1:
                # hop: row r becomes slot 0 of group g+1
                nc.sync.dma_start(
                    gap(Mt, p0 + S, S, 0, [[1, RS]]),
                    gap(Mt, p0, S, LR * RS, [[1, RS]]))

        def bwd_row(i):
            g, rho = i // LR, i % LR
            p0 = g * S
            nc.vector.tensor_tensor(
                gap(entB, p0, S, 0, [[1, Wb]]),
                gap(Nt, p0, S, (rho + 1) * RS, [[1, Wb]]),
                gap(Nt, p0, S, (rho + 1) * RS + 1, [[1, Wb]]),
                AluOpType.min)
            nc.vector.tensor_tensor_scan(
                gap(Nt, p0, S, rho * RS + Wb, [[-1, Wb]]),
                gap(entB, p0, S, Wb - 1, [[-1, Wb]]),
                gap(Dg, p0, S, rho * Wb + Wb - 1, [[-1, Wb]]),
                BIG, AluOpType.min, AluOpType.add)
            if rho == 0 and g > 0:
                # hop: row i becomes slot LR of group g-1
                nc.sync.dma_start(
                    gap(Nt, p0 - S, S, LR * RS, [[1, RS]]),
                    gap(Nt, p0, S, 0, [[1, RS]]))

        for k in range(N):
            fwd_row(k)
            bwd_row(N - 1 - k)

        # Y = womg - D on Pool (independent, runs during the DP)
        nc.gpsimd.tensor_tensor(
            gap(Yg, 0, NP, 0, [[Wb, LR], [1, Wb]]),
            gap(wom, 0, NP, 0, [[0, LR], [1, Wb]]),
            gap(Dg, 0, NP, 0, [[Wb, LR], [1, Wb]]),
            AluOpType.subtract)

        # ---- bias = +lam * M[N,N], replicated to all groups -----------
        p3 = (G - 1) * S
        nc.vector.tensor_scalar(
            bias[p3:NP, 0:1], gap(Mt, p3, S, LR * RS + W, [[1, 1]]),
            LAM, None, AluOpType.mult)
        for g in range(G - 1):
            nc.sync.dma_start(bias[g * S:(g + 1) * S, 0:1], bias[p3:NP, 0:1])
        nc.sync.dma_start(vals_d.ap(), gap(Mt, p3, S, LR * RS + W, [[1, 1]]))

        # ---- epilogue: X = M + num; X += Y; E*Om = Exp(-lam X + bias) -
        for ci in range(LR // ECH):
            c0 = ci * ECH
            xch = gap(Xg, 0, NP, c0 * Wb, [[Wb, ECH], [1, Wb]])
            nc.vector.tensor_tensor(
                xch,
                gap(Mt, 0, NP, (c0 + 1) * RS, [[RS, ECH], [1, Wb]]),
                gap(Nt, 0, NP, c0 * RS + 1, [[RS, ECH], [1, Wb]]),
                AluOpType.add)
            nc.vector.tensor_tensor(
                xch, xch, gap(Yg, 0, NP, c0 * Wb, [[Wb, ECH], [1, Wb]]),
                AluOpType.add)
            nc.scalar.activation(
                gap(Yg, 0, NP, c0 * Wb, [[Wb, ECH], [1, Wb]]), xch,
                ActivationFunctionType.Exp,
                bias=bias[:, 0:1], scale=-LAM,
                accum_out=tlp[:, ci:ci + 1])
        nc.vector.tensor_reduce(tls[:], tlp[:], mybir.AxisListType.X,
                                AluOpType.add)
        nc.sync.dma_start(tl_d.ap(), tls[:])

    nc.compile()
    return nc


_NC_CACHE = None


def _get_nc():
    global _NC_CACHE
    if _NC_CACHE is None:
        _NC_CACHE = _build_kernel()
    return _NC_CACHE


def _host_inputs(outputs, targets):
    """Full inputs -> per-core grouped/banded host arrays, concatenated."""
    outputs = np.asarray(outputs, np.float32)
    targets = np.asarray(targets, np.float32)
    B, T, C = outputs.shape
    t = np.ascontiguousarray(
        np.transpose(targets, (0, 2, 1)).reshape(B * C, T))
    o = np.ascontiguousarray(
        np.transpose(outputs, (0, 2, 1)).reshape(B * C, T))
    # grouped t: partition s+32g holds t[s, 32g:32g+32]
    tg = (t.reshape(N_CORES, S, G, LR).transpose(0, 2, 1, 3)
          .reshape(N_CORES * G * S, LR))
    # padded, grouped o: partition s+32g holds opad[s, 32g : 32g+OW]
    opad = np.full((B * C, T + 2 * W), SENT, np.float32)
    opad[:, W:W + T] = o
    og = np.empty((N_CORES, G, S, OW), np.float32)
    opad_c = opad.reshape(N_CORES, S, T + 2 * W)
    for g in range(G):
        og[:, g, :, :] = opad_c[:, :, g * LR:g * LR + OW]
    og = og.reshape(N_CORES * G * S, OW)
    return np.ascontiguousarray(tg), np.ascontiguousarray(og)


def _womg():
    k = np.arange(Wb, dtype=np.float64)
    om = (W - k) ** 2
    w = np.where(om == 0.0, BIG, -np.log(np.maximum(om, 1e-30)) / LAM)
    w = np.broadcast_to(w.astype(np.float32), (N_CORES * G * S, Wb))
    return np.ascontiguousarray(w)


_EXEC_CACHE = None


def _get_exec():
    """Build the sharded jitted executable once (mirrors bass2jax's
    run_bass_via_pjrt multi-core path)."""
    global _EXEC_CACHE
    if _EXEC_CACHE is not None:
        return _EXEC_CACHE
    import jax
    import concourse.mybir as _mybir
    from jax.sharding import Mesh, PartitionSpec, NamedSharding
    from jax.experimental.shard_map import shard_map
    from concourse.bass2jax import (
        _bass_exec_p, install_neuronx_cc_hook, partition_id_tensor)

    nc = _get_nc()
    install_neuronx_cc_hook()
    partition_name = nc.partition_id_tensor.name if nc.partition_id_tensor else None
    in_names, out_names, out_avals, zero_outs = [], [], [], []
    for alloc in nc.m.functions[0].allocations:
        if not isinstance(alloc, _mybir.MemoryLocationSet):
            continue
        name = alloc.memorylocations[0].name
        if alloc.kind == "ExternalInput":
            if name != partition_name:
                in_names.append(name)
        elif alloc.kind == "ExternalOutput":
            shape = tuple(alloc.tensor_shape)
            dtype = _mybir.dt.np(alloc.dtype)
            out_names.append(name)
            out_avals.append(jax.core.ShapedArray(shape, dtype))
            zero_outs.append(np.zeros(shape, dtype))
    n_params = len(in_names)
    all_in_names = list(in_names) + list(out_names)
    if partition_name is not None:
        all_in_names.append(partition_name)
    donate = tuple(range(n_params, n_params + len(out_names)))

    def _body(*args):
        operands = list(args)
        if partition_name is not None:
            operands.append(partition_id_tensor())
        return tuple(_bass_exec_p.bind(
            *operands,
            out_avals=tuple(out_avals),
            in_names=tuple(all_in_names),
            out_names=tuple(out_names),
            lowering_input_output_aliases=(),
            sim_require_finite=True,
            sim_require_nnan=True,
            nc=nc,
        ))

    devices = jax.devices()[:N_CORES]
    mesh = Mesh(np.asarray(devices), ("core",))
    in_specs = (PartitionSpec("core"),) * (n_params + len(out_names))
    out_specs = (PartitionSpec("core"),) * len(out_names)
    sharded = jax.jit(
        shard_map(_body, mesh=mesh, in_specs=in_specs, out_specs=out_specs,
                  check_rep=False),
        donate_argnums=donate, keep_unused=True)
    shard = NamedSharding(mesh, PartitionSpec("core"))
    wom_dev = jax.device_put(_womg(), shard)
    _EXEC_CACHE = (sharded, in_names, out_names, zero_outs, wom_dev)
    return _EXEC_CACHE


def kernel(outputs, targets):
    """outputs, targets: [64, 128, 4] float32 -> scalar float32 loss."""
    sharded, in_names, out_names, zero_outs, wom_dev = _get_exec()
    B, T, C = np.asarray(outputs).shape
    tg, og = _host_inputs(outputs, targets)
    by_name = {"tg": tg, "og": og, "wom": wom_dev}
    concat_in = [by_name[name] for name in in_names]
    concat_zeros = [
        np.zeros((N_CORES * z.shape[0], *z.shape[1:]), z.dtype)
        for z in zero_outs
    ]
    out_arrs = sharded(*concat_in, *concat_zeros)
    outs = {name: np.asarray(out_arrs[i]) for i, name in enumerate(out_names)}
    vals = outs["vals"][:, 0]
    tl = outs["tl"][:, 0]
    loss = 0.5 * (vals.sum(dtype=np.float64) / B) + \
           0.5 * (tl.sum(dtype=np.float64) / (B * T * T))
    return np.float32(loss)
